# revision 68
# baseline (speedup 1.0000x reference)
"""Single-head attention (SEQ=8192, EMBED=2048, HEAD=128) on 8 TRN2 NeuronCores.

Sharding: queries (rows of Q / the score matrix) split 1024 rows per core;
K and V are projected per-shard and exchanged with AllGathers.

Schedule (per core):
- Phase 1: x streams in as eight 128-row blocks over three DMA queues; x^T
  is built with PE transposes. K^T half-0 is projected first (output dtype
  fp8e4 — the wire and score-matmul format for K) and its AllGather fires;
  V blocks 0-3 follow (natural layout, bias via a contraction-1 ones-matmul
  so ScalarE stays free) so the CC stream runs K1, V1, K2, V2; then K
  half-1, Q (bias on ScalarE), V blocks 4-7.
- The 8 own K/V chunks are copied to slots 0-7 of the gathered buffers and
  their scores+exp+A@V run before any collective lands. Remote chunks are
  fetched from the AllGather outputs with per-core-rotated indirect-DMA
  gathers (idx rows from an ExternalInput), giving a uniform 56-slot remote
  stream with no core-dependent control flow.
- A warmup burst of wide matmuls, gated on a 64-byte DMA from the AG-K1
  output, un-throttles the PE clock (HAM) during the first unpack window.
- Phase 2 is an exp-throughput-bound stream: score units of 3 key-chunks
  ([128,3,512] f32 PSUM, double-buffered; 3 banks each) alternate between
  the two 512-query groups; each unit is one wide EXP on ScalarE (the only
  ACT work in the window). Softmax denominators accumulate on DVE as bf16
  tensor_tensor adds. A@V matmuls (fp8 K / bf16 V+pt) interleave into the
  stream's PE slack and accumulate into a [128,2,512] PSUM tile. Finals:
  ones-matmul row-sum, reciprocal, PE transposes, 1/l scaling, DMA out.

kernel(**inputs) takes the FULL unsharded inputs and returns the full output.
"""

import math

import numpy as np

import concourse.bacc as bacc
import concourse.bass as bass
import concourse.mybir as mybir
import concourse.tile as tile
from concourse.bass_utils import run_bass_kernel_spmd
from concourse.masks import make_identity

SEQ, EMBED, HEAD = 8192, 2048, 128
NCORES = 8
P = 128

F32 = mybir.dt.float32
BF16 = mybir.dt.bfloat16
F8 = mybir.dt.float8e4
U8 = mybir.dt.uint8
I32 = mybir.dt.int32

Id = mybir.ActivationFunctionType.Identity
Exp = mybir.ActivationFunctionType.Exp


def emit(nc, seq=SEQ, embed=EMBED, head=HEAD, ncores=NCORES):
    assert head == P
    s_loc = seq // ncores          # query rows per core (1024)
    e_ch = embed // P              # contraction chunks for projections (16)
    b_ch = s_loc // P              # 128-row blocks in local shard (8)
    n_half = s_loc // 2            # projection matmul free dim (512)
    sq_g = 512                     # phase-2 query group width
    n_g = s_loc // sq_g            # 2 groups
    n_t = seq // P                 # key/value chunks (64)
    scale = 1.0 / math.sqrt(head)

    x = nc.dram_tensor("x", [s_loc, embed], BF16, kind="ExternalInput").ap()
    wq = nc.dram_tensor("wq", [embed, head], BF16, kind="ExternalInput").ap()
    wk = nc.dram_tensor("wk", [embed, head], BF16, kind="ExternalInput").ap()
    wv = nc.dram_tensor("wv", [embed, head], BF16, kind="ExternalInput").ap()
    bq = nc.dram_tensor("bq", [head], F32, kind="ExternalInput").ap()
    bk = nc.dram_tensor("bk", [head], F32, kind="ExternalInput").ap()
    bv = nc.dram_tensor("bv", [head], F32, kind="ExternalInput").ap()
    # per-core gather rows: idx[p, i] = rot[i]*128 + p for the 7 remote ranks
    idxk = nc.dram_tensor("idxk", [P, ncores - 1], I32,
                          kind="ExternalInput").ap()
    out = nc.dram_tensor("out", [s_loc, head], F32, kind="ExternalOutput").ap()

    with tile.TileContext(nc) as tc:
        with (
            tc.tile_pool(name="consts", bufs=1) as consts,
            tc.tile_pool(name="persist", bufs=1) as persist,
            tc.tile_pool(name="dram", bufs=1, space="DRAM") as dram,
        ):
            ident = consts.tile([P, P], F32)
            make_identity(nc, ident)
            ident_bf = consts.tile([P, P], BF16)
            nc.vector.tensor_copy(ident_bf[:], ident[:])
            ones_f32 = consts.tile([P, 1], F32)
            nc.vector.memset(ones_f32[:], 1.0)
            ones_col = consts.tile([P, 1], BF16)
            nc.vector.tensor_copy(ones_col[:], ones_f32[:])
            ones_row = consts.tile([1, P], BF16)
            nc.vector.memset(ones_row[:], 1.0)

            # persistent SBUF across the whole kernel.
            # kt_sb/v_sb slot layout: slots 0-7 = OWN chunks (copied from
            # local projections, scoreable before any collective lands);
            # slots 8+i*4+j = remote gather position i (i-th of the 7 other
            # ranks, rotation order differs per core via idxk), wave-1 for
            # i<7, wave-2 at slots 36+.
            qt_sb = persist.tile([P, s_loc], BF16)           # Q^T own shard
            kt_sb = persist.tile([P, n_t, P], F8)            # K^T full (fp8)
            v_sb = persist.tile([P, n_t, P], BF16)           # V natural full
            idx_sb = persist.tile([P, ncores - 1], I32)
            # own chunks' exp'd scores live in persist so the early own-exp
            # doesn't wait for phase-1 SBUF (which phase-2's big pt reuses)
            pt_own = [persist.tile([P, 8, 512], BF16, name=f"pt_own{g}")
                      for g in range(2)]

            # K/V exchanged in halves (AG floor ~8us makes smaller slices
            # pointless; halves keep wave-2 from starving the exp stream).
            hsz1 = P * (s_loc // 2)
            hsz2 = P * (s_loc // 2)
            ag_k1_in = dram.tile([hsz1], F8)
            ag_k2_in = dram.tile([hsz2], F8)
            ag_v1_in = dram.tile([hsz1], BF16)
            ag_v2_in = dram.tile([hsz2], BF16)
            ag_k1_out = dram.tile([ncores * hsz1], F8, addr_space="Shared")
            ag_k2_out = dram.tile([ncores * hsz2], F8, addr_space="Shared")
            ag_v1_out = dram.tile([ncores * hsz1], BF16, addr_space="Shared")
            ag_v2_out = dram.tile([ncores * hsz2], BF16, addr_space="Shared")

            # ---------------- Phase 1: project own shard ----------------
            with (
                tc.tile_pool(name="p1", bufs=1) as p1,
                tc.tile_pool(name="trps", bufs=3, space="PSUM") as trps,
                tc.tile_pool(name="projps", bufs=2, space="PSUM") as projps,
            ):
                # x natural rows in, spread over four queues.
                x_b = x.rearrange("(b p) e -> b p e", p=P)
                x_sb = p1.tile([P, b_ch, embed], BF16)
                x_eng = [nc.sync, nc.gpsimd, nc.scalar]
                for b in range(b_ch):
                    x_eng[b % 3].dma_start(x_sb[:, b, :], x_b[b])

                bq_sb = p1.tile([P, 1], F32)
                bk_sb = p1.tile([P, 1], F32)
                bv_sb = p1.tile([1, P], F32)
                nc.scalar.dma_start(bq_sb[:], bq.unsqueeze(1))
                nc.scalar.dma_start(bk_sb[:], bk.unsqueeze(1))
                nc.scalar.dma_start(bv_sb[:], bv.unsqueeze(0))
                nc.scalar.dma_start(idx_sb[:], idxk)

                wq_sb = p1.tile([P, e_ch, head], BF16)
                wk_sb = p1.tile([P, e_ch, head], BF16)
                wv_sb = p1.tile([P, e_ch, head], BF16)
                nc.scalar.dma_start(
                    wk_sb[:], wk.rearrange("(c p) h -> p c h", p=P))
                nc.sync.dma_start(
                    wq_sb[:], wq.rearrange("(c p) h -> p c h", p=P))
                nc.gpsimd.dma_start(
                    wv_sb[:], wv.rearrange("(c p) h -> p c h", p=P))
                bv_bf = p1.tile([1, P], BF16)
                nc.scalar.copy(bv_bf[:], bv_sb[:])

                xt = p1.tile([P, e_ch, s_loc], BF16)
                kt_loc = p1.tile([P, s_loc], F8)
                v_nat = p1.tile([P, b_ch, head], BF16)

                def transpose_blocks(b0, b1):
                    for b in range(b0, b1):
                        for eq in range(0, e_ch, 4):
                            tr = trps.tile([P, 4, P], BF16, tag="tr")
                            for j in range(4):
                                e = eq + j
                                nc.tensor.transpose(
                                    tr[:, j, :],
                                    x_sb[:, b, e * P:(e + 1) * P],
                                    ident_bf[:])
                            dst = xt[:, eq:eq + 4, b * P:(b + 1) * P]
                            if (b + eq // 4) % 2 == 0:
                                nc.vector.tensor_copy(dst, tr[:])
                            else:
                                nc.scalar.copy(dst, tr[:])

                def project(w_sb, b_sb, dst, c0, c1, tag="pps"):
                    hsl = slice(c0, c1)
                    ps = projps.tile([P, n_half], F32, tag=tag)
                    for e in range(e_ch):
                        nc.tensor.matmul(
                            ps[:, 0:c1 - c0], w_sb[:, e, :], xt[:, e, hsl],
                            start=(e == 0), stop=(e == e_ch - 1))
                    nc.scalar.activation(dst[:, hsl], ps[:, 0:c1 - c0], Id,
                                         bias=b_sb[:, 0:1])

                def fire_ag(ag_in, ag_out):
                    nc.gpsimd.collective_compute(
                        "AllGather", mybir.AluOpType.bypass,
                        replica_groups=[list(range(ncores))],
                        ins=[ag_in.opt()], outs=[ag_out.opt()])

                # V directly in natural layout: per 128-row block, 16
                # accumulating matmuls (stationary = x^T e-chunk) plus a
                # contraction-1 ones-matmul that broadcasts the bias.
                def v_blocks(b0, b1):
                    for b in range(b0, b1):
                        vps = projps.tile([P, head], F32, tag="vps")
                        for e in range(e_ch):
                            nc.tensor.matmul(
                                vps[:], xt[:, e, b * P:(b + 1) * P],
                                wv_sb[:, e, :],
                                start=(e == 0), stop=False)
                        nc.tensor.matmul(
                            vps[:], ones_row[:], bv_bf[:],
                            start=False, stop=True)
                        nc.vector.tensor_copy(v_nat[:, b, :], vps[:])

                # CC stream order: K1, V1, K2, V2. V blocks 0-3 are
                # projected right after K-half-0 (they only need x^T blocks
                # 0-3) so V1's data is genuinely ready before K2's and the
                # scheduler orders the triggers K1, V1, K2, V2; V1 landing
                # second lets the A@V stream start without stalling the
                # in-order PE queue. V wire layout is p-major so K and V
                # share the same 512-element gather-row geometry.
                transpose_blocks(0, 4)
                project(wk_sb, bk_sb, kt_loc, 0, n_half)
                nc.sync.dma_start(
                    ag_k1_in.rearrange("(p s) -> p s", p=P),
                    kt_loc[:, 0:n_half])
                fire_ag(ag_k1_in, ag_k1_out)
                v_blocks(0, 4)
                nc.sync.dma_start(
                    ag_v1_in.rearrange("(p b h) -> p b h", p=P, h=head),
                    v_nat[:, 0:4, :])
                fire_ag(ag_v1_in, ag_v1_out)
                transpose_blocks(4, 8)
                project(wk_sb, bk_sb, kt_loc, n_half, s_loc)
                nc.sync.dma_start(
                    ag_k2_in.rearrange("(p s) -> p s", p=P),
                    kt_loc[:, n_half:s_loc])
                fire_ag(ag_k2_in, ag_k2_out)

                # Q next: it gates the score stream.
                project(wq_sb, bq_sb, qt_sb, 0, n_half)
                project(wq_sb, bq_sb, qt_sb, n_half, s_loc)

                v_blocks(4, b_ch)
                nc.sync.dma_start(
                    ag_v2_in.rearrange("(p b h) -> p b h", p=P, h=head),
                    v_nat[:, 4:b_ch, :])
                fire_ag(ag_v2_in, ag_v2_out)

                # own chunks into slots 0-7 of the uniform slot layout
                nc.vector.tensor_copy(
                    kt_sb[:, 0:b_ch, :], kt_loc.rearrange(
                        "p (b t) -> p b t", t=P))
                nc.vector.tensor_copy(v_sb[:, 0:b_ch, :], v_nat[:])

            # unpack gathered K^T / V into SBUF. Slot s in kt_sb and v_sb
            # refer to the same original chunk: wave-1 slots r*2+j (j<2),
            # wave-2 slots 16 + r*6 + j.
            # HAM warmup gate: a tiny DMA that lands the moment AG-K1
            # completes; the phase-2 warmup matmuls read it, so they run
            # during the unpack window and un-throttle the PE clock before
            # the score stream starts (PE sat idle >3.4us waiting for the
            # collective, so it enters phase 2 at K=4/8 otherwise).
            warm_sb = persist.tile([P, P], F8)
            nc.sync.dma_start(warm_sb[0:1, 0:64], ag_k1_out[0:64].unsqueeze(0))

            # unpack: per-core-rotated gathers (one indirect DMA per remote
            # rank position, so position 0 lands first and the stream starts
            # without waiting for the whole wave). Row index = rank*128 + p.
            def gather_pos(ag_out, dst, dst0, i):
                src = ag_out.rearrange("(r s) -> r s", s=512)
                # dest free dims must be flat: the DGE derives the
                # per-index run length from the dest innermost dim.
                d = dst[:, dst0 + i * 4:dst0 + (i + 1) * 4, :].rearrange(
                    "p b t -> p (b t)")
                nc.gpsimd.indirect_dma_start(
                    out=d,
                    out_offset=None,
                    in_=src,
                    in_offset=bass.IndirectOffsetOnAxis(
                        ap=idx_sb[:, i:i + 1], axis=0),
                )

            # wave 1: V1's gather calls interleave into K1's so the A@V
            # dependencies land ~5us earlier; K1 positions have slack
            # (landing pitch 1.4us vs consumption pitch 4.2us).
            # k2's first calls interleave into v1's tail (and v2's into
            # k2's) so each wave's position-0 fires at its completion sem
            # instead of queuing behind the previous wave's descgen.
            order = [("k1", 0), ("k1", 1), ("k1", 2), ("v1", 0), ("k1", 3),
                     ("v1", 1), ("k1", 4), ("v1", 2), ("k1", 5), ("v1", 3),
                     ("k1", 6), ("v1", 4), ("k2", 0), ("v1", 5), ("k2", 1),
                     ("v1", 6), ("k2", 2), ("k2", 3), ("k2", 4), ("v2", 0),
                     ("k2", 5), ("v2", 1), ("k2", 6), ("v2", 2), ("v2", 3),
                     ("v2", 4), ("v2", 5), ("v2", 6)]
            waves = {"k1": (ag_k1_out, kt_sb, 8), "v1": (ag_v1_out, v_sb, 8),
                     "k2": (ag_k2_out, kt_sb, 36), "v2": (ag_v2_out, v_sb, 36)}
            for kind, i in order:
                ag_out, dst, dst0 = waves[kind]
                gather_pos(ag_out, dst, dst0, i)

            # ---------------- Phase 2: attention ----------------
            # Own chunks (slots 0-7) are scored and exp'd around t~35us,
            # long before any collective lands; the remote stream (56
            # chunks) starts when the first K gather position arrives.
            own_units = [(0, 3), (3, 3), (6, 2)]
            rem_units = [(8 + 3 * k, 3) for k in range(18)] + [(62, 2)]
            with (
                tc.tile_pool(name="p2", bufs=1) as p2,
                tc.tile_pool(name="p2s", bufs=2) as p2s,
                tc.tile_pool(name="stps", bufs=2, space="PSUM") as stps,
                tc.tile_pool(name="avps", bufs=1, space="PSUM") as avps,
            ):
                pt = [p2.tile([P, n_t, sq_g], BF16, name=f"pt{g}")
                      for g in range(n_g)]

                def pt_sl(g, c0, w):
                    if c0 < 8:
                        return pt_own[g][:, c0:c0 + w, :]
                    return pt[g][:, c0:c0 + w, :]
                acc = [p2.tile([P, 3, sq_g], BF16, name=f"acc{g}")
                       for g in range(n_g)]
                avt = avps.tile([P, n_g, sq_g], F32)

                # A@V jobs interleave into the score stream's PE slack (each
                # exp takes ~1.6us vs ~0.65us of score matmuls): slots 0-39
                # (both groups) ride unit-pairs 8..17; the rest ride the
                # group-serial tail blocks so each group's AV finishes with
                # its exps and finals(g0) overlaps g1's last exps.
                av_jobs = [(s, g) for s in range(8, n_t) for g in range(n_g)]

                def av_emit(n):
                    for _ in range(n):
                        if not av_jobs:
                            return
                        s, g = av_jobs.pop(0)
                        nc.tensor.matmul(
                            avt[:, g, :], v_sb[:, s, :], pt_sl(g, s, 1)[:, 0, :],
                            start=(s == 0), stop=(s == n_t - 1),
                            skip_group_check=True)

                def unit(g, c0, w):
                    qg = qt_sb[:, g * sq_g:(g + 1) * sq_g]
                    st = stps.tile([P, 3, sq_g], F32, tag="st")
                    for k in range(w):
                        nc.tensor.matmul(
                            st[:, k, :], kt_sb[:, c0 + k, :], qg,
                            start=True, stop=True, skip_group_check=True)
                    nc.scalar.activation(
                        pt_sl(g, c0, w), st[:, 0:w, :], Exp,
                        scale=scale)
                    if c0 == 3:
                        nc.vector.tensor_tensor(
                            acc[g][:], pt_sl(g, 0, 3),
                            pt_sl(g, 3, 3), mybir.AluOpType.add)
                    elif c0 > 3:
                        nc.vector.tensor_tensor(
                            acc[g][:, 0:w, :], acc[g][:, 0:w, :],
                            pt_sl(g, c0, w),
                            mybir.AluOpType.add)

                fin_state = {}

                def finals_a(g):
                    # denominator fold + copies; group 1's DVE prep overlaps
                    # group 0's PE transpose half below.
                    nc.vector.tensor_tensor(
                        acc[g][:, 0:1, :], acc[g][:, 0:1, :],
                        acc[g][:, 1:2, :], mybir.AluOpType.add)
                    nc.vector.tensor_tensor(
                        acc[g][:, 0:1, :], acc[g][:, 0:1, :],
                        acc[g][:, 2:3, :], mybir.AluOpType.add)
                    l_ps = stps.tile([1, sq_g], F32, tag="st", name="l_ps")
                    nc.tensor.matmul(
                        l_ps[:], ones_col[:], acc[g][:, 0, :],
                        start=True, stop=True, skip_group_check=True)
                    l_sb = p2s.tile([1, sq_g], F32, tag="lsb")
                    nc.vector.tensor_copy(l_sb[:], l_ps[:])
                    ot_sb = p2s.tile([P, sq_g], F32, tag="otsb")
                    nc.vector.tensor_copy(ot_sb[:], avt[:, g, :])
                    fin_state[g] = (l_sb, ot_sb)

                def finals_b(g):
                    l_sb, ot_sb = fin_state[g]
                    lc_ps = stps.tile([P, 4, 1], F32, tag="st", name="lc_ps")
                    for j in range(sq_g // P):
                        nc.tensor.transpose(
                            lc_ps[:, j, :], l_sb[0:1, j * P:(j + 1) * P],
                            ident[0:1, 0:1])
                    r_col = p2s.tile([P, 4, 1], F32, tag="rcol")
                    nc.vector.reciprocal(r_col[:], lc_ps[:])
                    for j in range(sq_g // P):
                        o_tr = stps.tile([P, P], F32, tag="st", name="o_tr")
                        nc.tensor.transpose(
                            o_tr[:], ot_sb[:, j * P:(j + 1) * P],
                            ident[:])
                        o_sb = p2s.tile([P, head], F32, tag="osb")
                        nc.vector.tensor_scalar_mul(
                            o_sb[:], o_tr[:], r_col[:, j, 0:1])
                        row0 = g * sq_g + j * P
                        eng = nc.sync if j % 2 == 0 else nc.scalar
                        eng.dma_start(out[row0:row0 + P, :], o_sb[:])

                # own chunks: scores + exp + AV, all pre-collective
                for c0, w in own_units:
                    for g in range(n_g):
                        unit(g, c0, w)
                for s in range(8):
                    for g in range(n_g):
                        nc.tensor.matmul(
                            avt[:, g, :], v_sb[:, s, :],
                            pt_own[g][:, s, :],
                            start=(s == 0), stop=False,
                            skip_group_check=True)

                # warmup burst: ~3.4us of gap-free wide matmuls reading the
                # AG-K1 completion gate; flips HAM to K=8/8 during the
                # unpack window (un-throttle needs a contiguous busy window;
                # the PE sat idle waiting for the collective).
                for _ in range(8):
                    wt = stps.tile([P, 512], F32, tag="st", name="warm_ps")
                    nc.tensor.matmul(
                        wt[:], warm_sb[:], qt_sb[:, 0:512],
                        start=True, stop=True, skip_group_check=True)

                for ui, (c0, w) in enumerate(rem_units):
                    for g in range(n_g):
                        unit(g, c0, w)
                    # 6/pair fits the per-pair PE budget even when the SW
                    # thermal throttle caps the PE at ~1.95GHz; more would
                    # make the stream PE-bound and stretch the exp pace.
                    if ui >= 6:
                        av_emit(6)
                av_emit(len(av_jobs))
                finals_a(0)
                finals_a(1)
                finals_b(0)
                finals_b(1)
    nc.compile()
    return nc


_CACHE = {}


def _get_nc():
    if "nc" not in _CACHE:
        nc = bacc.Bacc("TRN2", target_bir_lowering=False, debug=False,
                       num_devices=NCORES)
        _CACHE["nc"] = emit(nc)
    return _CACHE["nc"]


def make_in_maps(x, Wq, bq, Wk, bk, Wv, bv):
    import ml_dtypes
    bf = ml_dtypes.bfloat16
    x = np.ascontiguousarray(np.asarray(x, dtype=np.float32).astype(bf))
    Wq = np.ascontiguousarray(np.asarray(Wq, dtype=np.float32).astype(bf))
    Wk = np.ascontiguousarray(np.asarray(Wk, dtype=np.float32).astype(bf))
    Wv = np.ascontiguousarray(np.asarray(Wv, dtype=np.float32).astype(bf))
    bq = np.ascontiguousarray(np.asarray(bq, dtype=np.float32))
    bk = np.ascontiguousarray(np.asarray(bk, dtype=np.float32))
    bv = np.ascontiguousarray(np.asarray(bv, dtype=np.float32))
    s_loc = SEQ // NCORES
    maps = []
    for c in range(NCORES):
        rot = [r for r in range(NCORES) if r != c]
        idxk = np.array(
            [[r * 128 + p for r in rot] for p in range(128)], dtype=np.int32)
        maps.append({
            "x": np.ascontiguousarray(x[c * s_loc:(c + 1) * s_loc]),
            "wq": Wq, "wk": Wk, "wv": Wv,
            "bq": bq, "bk": bk, "bv": bv,
            "idxk": np.ascontiguousarray(idxk),
        })
    return maps


def kernel(x, Wq, bq, Wk, bk, Wv, bv):
    in_maps = make_in_maps(x, Wq, bq, Wk, bk, Wv, bv)
    res = run_bass_kernel_spmd(_get_nc(), in_maps,
                               core_ids=list(range(NCORES)))
    return np.concatenate(
        [res.results[c]["out"] for c in range(NCORES)], axis=0)
